# revision 8
# baseline (speedup 1.0000x reference)
"""MedianPool2d 3x3 stride-1 reflect-pad kernel for 8 TRN2 NeuronCores.

Input:  x [16, 3, 512, 512] fp32 (full). Output: same shape, lower median
of each 3x3 window after reflect pad (exact order statistic -> bitwise
exact vs reference).

Strategy:
 - Pure data parallel: 48 images (B*C) -> 6 images per core.
 - Host-side staging: per core, each of 3 tiles holds 2 images split
   across 128 partitions; partition p carries 8 output rows plus its
   2 halo rows and reflect-padded columns, flattened to 10*514 fp32.
   Both vertical (+-514) and horizontal (+-1) window shifts then become
   free-dim offsets of one flat SBUF buffer, and each tile needs exactly
   one input DMA.
 - Median-of-9 via the exact identity
       med9 = med3( max3(col mins), med3(col meds), min3(col maxes) )
   computed with 18 full-tile min/max tensor_tensor ops, statically
   split across DVE (10) and GpSimd/Pool (8).
"""

import sys

for _p in ("/opt/trn_rl_repo", "/root/.axon_site/_ro/trn_rl_repo"):
    if _p not in sys.path:
        sys.path.append(_p)

import numpy as np

import concourse.bass as bass
import concourse.bacc as bacc
import concourse.mybir as mybir
from concourse.tile import TileContext

F32 = mybir.dt.float32
MIN = mybir.AluOpType.min
MAX = mybir.AluOpType.max

ROWS_PER_CORE = 3072  # 6 images x 512 rows
W = 512
WP = 514  # padded row width
RPP = 8  # image rows per partition
NSLOT = RPP + 2  # + top/bottom halo rows
FLAT = NSLOT * WP  # 5140 floats per partition in the staged input
CLEN = RPP * WP  # 4112: flat length of per-position stats / outputs
N_TILES = 3  # 2 images per tile
ROWS_PER_TILE = 1024

_NC_CACHE = None


def _build_bass(loop_k=1):
    nc = bacc.Bacc("TRN2", target_bir_lowering=False)
    x_d = nc.declare_dram_parameter("x", [N_TILES, 128, FLAT], F32, isOutput=False)
    o_d = nc.declare_dram_parameter("out", [ROWS_PER_CORE, W], F32, isOutput=True)

    import contextlib
    with TileContext(nc) as tc:
        loop_cm = tc.For_i(0, loop_k, 1) if loop_k > 1 else contextlib.nullcontext()
        with loop_cm, tc.tile_pool(name="pool", bufs=1) as pool:
            for t in range(N_TILES):
                r0 = t * ROWS_PER_TILE
                xin = pool.tile([128, FLAT], F32, tag="xin", bufs=3)
                nc.sync.dma_start(out=xin[:], in_=x_d[t])

                xf = xin[:]
                v0 = xf[:, 0:CLEN]
                v1 = xf[:, WP : WP + CLEN]
                v2 = xf[:, 2 * WP : 2 * WP + CLEN]

                P1 = pool.tile([128, CLEN], F32, tag="p1")
                P2 = pool.tile([128, CLEN], F32, tag="p2")
                S1 = pool.tile([128, CLEN], F32, tag="s1")
                S2 = pool.tile([128, CLEN], F32, tag="s2")
                S3 = pool.tile([128, CLEN], F32, tag="s3")
                T1 = pool.tile([128, CLEN], F32, tag="t1")
                O = pool.tile([128, CLEN], F32, tag="o", bufs=2)

                # column stage: per-position vertical min/med/max.
                # All xin readers stay on DVE so the input DMA's slot-reuse
                # wait collapses to one engine semaphore (DMA sync-wait
                # count is tightly limited in codegen).
                nc.vector.tensor_tensor(P1[:], v0, v1, MIN)  # pair min
                nc.vector.tensor_tensor(P2[:], v0, v1, MAX)  # pair max
                nc.vector.tensor_tensor(S1[:], P1[:], v2, MIN)  # cmin
                nc.vector.tensor_tensor(S2[:], P2[:], v2, MAX)  # cmax
                nc.vector.tensor_tensor(P2[:], P2[:], v2, MIN)  # t5 (in-place)
                nc.vector.tensor_tensor(S3[:], P1[:], P2[:], MAX)  # cmed

                c = slice(1, CLEN - 1)
                l = slice(0, CLEN - 2)
                r = slice(2, CLEN)

                # A = max3(cmin left, center, right)
                nc.vector.tensor_tensor(P1[:, c], S1[:, l], S1[:, r], MAX)
                nc.vector.tensor_tensor(P1[:, c], P1[:, c], S1[:, c], MAX)  # A
                # C = min3(cmax)
                nc.vector.tensor_tensor(T1[:, c], S2[:, l], S2[:, r], MIN)
                nc.vector.tensor_tensor(T1[:, c], T1[:, c], S2[:, c], MIN)  # C
                # B = med3(cmed l, c, r)
                nc.vector.tensor_tensor(S1[:, c], S3[:, l], S3[:, c], MIN)  # m1
                nc.vector.tensor_tensor(S2[:, c], S3[:, l], S3[:, c], MAX)  # m2
                nc.vector.tensor_tensor(S2[:, c], S2[:, c], S3[:, r], MIN)  # m3
                nc.vector.tensor_tensor(S1[:, c], S1[:, c], S2[:, c], MAX)  # B
                # out = med3(A=P1, B=S1, C=T1)
                nc.vector.tensor_tensor(S3[:, c], P1[:, c], S1[:, c], MIN)  # mn2
                nc.vector.tensor_tensor(P2[:, c], P1[:, c], S1[:, c], MAX)  # mx2
                nc.vector.tensor_tensor(P2[:, c], P2[:, c], T1[:, c], MIN)  # t3
                nc.vector.tensor_tensor(O[:, c], S3[:, c], P2[:, c], MAX)  # median

                # store: rows y cols 1..512 of each padded row
                o3 = O[:].rearrange("p (s w) -> p s w", w=WP)
                dst = o_d[r0 : r0 + ROWS_PER_TILE].rearrange("(p s) w -> p s w", s=RPP)
                nc.sync.dma_start(out=dst, in_=o3[:, :, 1 : W + 1])
    return nc


def _get_nc():
    global _NC_CACHE
    if _NC_CACHE is None:
        nc = _build_bass()
        nc.compile()
        _NC_CACHE = nc
    return _NC_CACHE


def _stage_core(imgs):
    """imgs: [6, 512, 512] fp32 -> staged [3, 128, FLAT] with halo rows and
    reflect-padded rows/cols materialized."""
    xp = np.pad(imgs, ((0, 0), (1, 1), (1, 1)), mode="reflect")  # [6, 514, 514]
    # windows of 10 padded rows starting every 8 rows: [6, 64, 10, 514]
    win = np.lib.stride_tricks.sliding_window_view(xp, (NSLOT, WP), axis=(1, 2))
    blocks = win[:, ::RPP, 0]  # [6, 64, 10, 514]
    staged = blocks.reshape(N_TILES, 128, NSLOT, WP).reshape(N_TILES, 128, FLAT)
    return np.ascontiguousarray(staged)


def run(x, trace=False):
    """x: [16,3,512,512] fp32 -> (out [16,3,512,512] fp32, exec_time_ns|None)"""
    from concourse.bass_utils import run_bass_kernel_spmd

    x = np.ascontiguousarray(np.asarray(x, dtype=np.float32))
    B, C, H, Wd = x.shape
    imgs = x.reshape(8, 6, H, Wd)
    in_maps = [{"x": _stage_core(imgs[i])} for i in range(8)]
    nc = _get_nc()
    res = run_bass_kernel_spmd(nc, in_maps, list(range(8)), trace=trace)
    out = np.stack([res.results[i]["out"] for i in range(8)])
    return out.reshape(B, C, H, Wd), res.exec_time_ns


def kernel(x):
    out, _ = run(x, trace=False)
    return out
